# revision 1
# baseline (speedup 1.0000x reference)
"""CrossAttention kernel for 8 Trainium2 NeuronCores.

Sharding: data-parallel over batch (4) x tensor-parallel over head pairs (2).
Core c handles batch b=c//2 and heads [4g, 4g+4) with g=c%2.
Each core computes LN(target_b) once, per-head Q/K/V projections, the bilinear
K transform, softmax attention (no max-subtraction: logits are ~N(0, 0.017)),
ELU via the exact identity elu(x) = relu(x) + min(exp(x),1) - 1, and a partial
W_O matmul; a pairwise ReduceScatter sums the W_O partials and leaves each core
with its half of the rows, to which it adds the residual.

Matmuls run in bf16 (fp32 accumulate in PSUM); LN, softmax normalization,
ELU arithmetic, and the residual stay in fp32.
"""
import math
import sys

sys.path.insert(0, "/opt/trn_rl_repo")

import ml_dtypes
import numpy as np

import concourse.bass as bass
import concourse.mybir as mybir
import concourse.tile as tile
from concourse.bass_utils import run_bass_kernel_spmd
from concourse.masks import make_identity
from concourse.vector_clock import ScopedClock

B, N, P, C, H = 4, 1024, 1024, 512, 8
HL = H // 2          # heads per core
CT = C // 128        # 4 contraction tiles
NT = N // 128        # 8 row tiles
F32 = mybir.dt.float32
BF16 = mybir.dt.bfloat16
AF = mybir.ActivationFunctionType
ALU = mybir.AluOpType
INV_C = 1.0 / C      # the two 1/sqrt(C) softmax scales combined


# --- walrus on this container allows a single sync-wait per CTRL_NO (Drain)
# --- instruction; Tile's kernel-tail drain aggregates one wait per engine/DMA
# --- lane. Split them across a chain of drains, one wait each.
def _patched_drain_and_barrier(self, tick_clock, wait_clock):
    drain_inst = self.nc.sync.drain()
    wait_clock.add_sem_waits(
        drain_inst.ins, ScopedClock({None: tick_clock.global_clock})
    )
    ins = drain_inst.ins
    waits = list(ins.sync_info.on_wait) if (ins.sync_info and ins.sync_info.on_wait) else []
    if len(waits) > 1:
        ins.sync_info.on_wait = waits[:1]
        for i in range(1, len(waits)):
            extra = self.nc.sync.drain()
            si = extra.ins.sync_info
            if si is None:
                extra.ins.sync_info = mybir.SyncInfo(on_wait=[waits[i]], on_update=[])
            else:
                si.on_wait = [waits[i]]
    self.nc.all_engine_barrier()
    popped = self.nc._tile_sem_poison_stack.pop()
    assert popped is self._sem_poison
    self.nc.clear_and_free_semaphores(list(self.sems.allocated().values()))
    self.nc.all_engine_barrier()


tile.TileContext._drain_and_barrier = _patched_drain_and_barrier


# --- same single-wait rule applies to every ISA struct on this walrus
# --- (TensorTensor/Activation/Matmult/DMACopy all reject >=2 sync waits).
# --- Split excess waits onto injected NOPs on the same engine: engine FIFO
# --- order makes the NOP's wait happen-before the real instruction.
_orig_commit = tile.TileContext._commit_instruction


def _patched_commit(self, inst, lazy_reg_writes=True):
    si = getattr(inst, "sync_info", None)
    if si is not None and si.on_wait and len(si.on_wait) > 1 \
            and inst.engine != mybir.EngineType.Unassigned:
        waits = list(si.on_wait)
        si.on_wait = waits[:1]
        for w in waits[1:]:
            nop = mybir.InstNoOp(name=self.nc.get_next_instruction_name())
            nop.engine = inst.engine
            nop.sync_info = mybir.SyncInfo(on_wait=[w], on_update=[])
            _orig_commit(self, nop, lazy_reg_writes=False)
    return _orig_commit(self, inst, lazy_reg_writes)


tile.TileContext._commit_instruction = _patched_commit


def _r(ap):
    """[R*128, F] dram view -> [128, R, F] (partition, row-tile, free)."""
    return ap.rearrange("(t p) f -> p t f", p=128)


def build():
    nc = bass.Bass()
    target = nc.declare_dram_parameter("target", [N, C], F32, isOutput=False)
    resid = nc.declare_dram_parameter("resid", [N // 2, C], F32, isOutput=False)
    src = nc.declare_dram_parameter("src_bf", [P, C], BF16, isOutput=False)
    ln_g = nc.declare_dram_parameter("ln_g", [C], F32, isOutput=False)
    ln_b = nc.declare_dram_parameter("ln_b", [C], F32, isOutput=False)
    wq_d = nc.declare_dram_parameter("wq", [C, HL * C], BF16, isOutput=False)
    wk_d = nc.declare_dram_parameter("wk", [C, HL * C], BF16, isOutput=False)
    wv_d = nc.declare_dram_parameter("wv", [C, HL * C], BF16, isOutput=False)
    watt_d = nc.declare_dram_parameter("watt", [HL, C, C], BF16, isOutput=False)
    wo_d = nc.declare_dram_parameter("wo", [HL * C, C], BF16, isOutput=False)
    out_d = nc.declare_dram_parameter("out", [N // 2, C], F32, isOutput=True)

    with tile.TileContext(nc) as tc, \
         tc.tile_pool(name="singles", bufs=1) as sg, \
         tc.tile_pool(name="wpool", bufs=10) as wp, \
         tc.tile_pool(name="acts", bufs=1) as acts, \
         tc.tile_pool(name="small", bufs=2) as sm, \
         tc.tile_pool(name="ps", bufs=5, space="PSUM") as ps, \
         tc.tile_pool(name="dram", bufs=1, space="DRAM") as dram:

        # ---------- phase 0: constants, LN, transposes ----------
        ident = sg.tile([128, 128], BF16)
        make_identity(nc, ident)
        ones_col = sg.tile([128, 1], BF16)
        nc.vector.memset(ones_col, 1.0)
        ones_row = sg.tile([1, 128], BF16)
        nc.vector.memset(ones_row, 1.0)
        eps_t = sg.tile([128, 1], F32)
        nc.vector.memset(eps_t, 1e-5)
        g_bc = sg.tile([128, C], F32)
        nc.gpsimd.dma_start(out=g_bc, in_=ln_g[None, :].to_broadcast([128, C]))
        b_bc = sg.tile([128, C], F32)
        nc.gpsimd.dma_start(out=b_bc, in_=ln_b[None, :].to_broadcast([128, C]))

        x_nat = sg.tile([128, NT, C], F32)          # target, natural layout
        nc.sync.dma_start(out=x_nat, in_=_r(target[:]))
        t_bf = sg.tile([128, NT, C], BF16)          # LN output, bf16
        sT = sg.tile([128, CT, P], BF16)            # source^T
        tT = sg.tile([128, CT, N], BF16)            # LN(target)^T
        wo_acc = sg.tile([128, NT, C], F32)         # W_O partial accumulator

        # source^T via DMA transpose straight from DRAM
        for ct in range(CT):
            nc.sync.dma_start(out=sT[:, ct, :], in_=src[:, ct * 128:(ct + 1) * 128],
                              transpose=True)

        # LayerNorm on each row-tile of target
        for nt in range(NT):
            stats = sm.tile([128, 6], F32, tag="stats")
            nc.vector.bn_stats(out=stats, in_=x_nat[:, nt, :])
            mv = sm.tile([128, 2], F32, tag="mv", bufs=NT)
            nc.vector.bn_aggr(out=mv, in_=stats)
            rstd = sm.tile([128, 1], F32, tag="rstd", bufs=NT)
            nc.scalar.activation(rstd, mv[:, 1:2], AF.Sqrt, bias=eps_t, scale=1.0)
            nc.vector.reciprocal(out=rstd, in_=rstd)
            t0 = sm.tile([128, C], F32, tag="t0")
            nc.vector.tensor_scalar(t0, x_nat[:, nt, :], mv[:, 0:1], rstd,
                                    op0=ALU.subtract, op1=ALU.mult)
            t1 = sm.tile([128, C], F32, tag="t1")
            nc.vector.tensor_mul(t1, t0, g_bc)
            nc.vector.tensor_add(t_bf[:, nt, :], t1, b_bc)

        # t^T via PE transpose of 128x128 blocks
        for ct in range(CT):
            for ng in range(2):
                ptr = ps.tile([128, 4, 128], BF16, tag="tr", bufs=1)
                for j in range(4):
                    nt = ng * 4 + j
                    nc.tensor.transpose(ptr[:, j, :], t_bf[:, nt, ct * 128:(ct + 1) * 128],
                                        ident)
                nc.scalar.copy(tT[:, ct, ng * 512:(ng + 1) * 512], ptr)

        # ---------- per-head pipeline ----------
        for h in range(HL):
            hs = slice(h * C, (h + 1) * C)
            wq_h = wp.tile([128, CT, C], BF16, tag="w", name=f"wq{h}")
            nc.sync.dma_start(out=wq_h, in_=_r(wq_d[:, hs]))
            wk_h = wp.tile([128, CT, C], BF16, tag="w", name=f"wk{h}")
            nc.sync.dma_start(out=wk_h, in_=_r(wk_d[:, hs]))
            wv_h = wp.tile([128, CT, C], BF16, tag="w", name=f"wv{h}")
            nc.sync.dma_start(out=wv_h, in_=_r(wv_d[:, hs]))
            wa_h = wp.tile([128, CT, C], BF16, tag="w", name=f"wa{h}")
            nc.sync.dma_start(out=wa_h, in_=_r(watt_d[h]))
            wo_h = wp.tile([128, CT, C], BF16, tag="w", name=f"wo{h}")
            nc.sync.dma_start(out=wo_h, in_=_r(wo_d[hs, :]))

            # qT[d, n] = sum_c Wq[c, d] * tT[c, n]
            qT = acts.tile([128, CT, N], BF16, tag="qT", bufs=2, name=f"qT{h}")
            k0T = acts.tile([128, CT, P], BF16, tag="k0T", name=f"k0T{h}")
            for dt in range(CT):
                for nch in range(2):
                    pq = ps.tile([128, 512], F32, tag="mm", name=f"pq{h}{dt}{nch}")
                    for ct in range(CT):
                        nc.tensor.matmul(pq, wq_h[:, ct, dt * 128:(dt + 1) * 128],
                                         tT[:, ct, nch * 512:(nch + 1) * 512],
                                         start=(ct == 0), stop=(ct == CT - 1))
                    nc.vector.tensor_copy(qT[:, dt, nch * 512:(nch + 1) * 512], pq)
                    pk0 = ps.tile([128, 512], F32, tag="mm", name=f"pk0{h}{dt}{nch}")
                    for ct in range(CT):
                        nc.tensor.matmul(pk0, wk_h[:, ct, dt * 128:(dt + 1) * 128],
                                         sT[:, ct, nch * 512:(nch + 1) * 512],
                                         start=(ct == 0), stop=(ct == CT - 1))
                    nc.vector.tensor_copy(k0T[:, dt, nch * 512:(nch + 1) * 512], pk0)

            # kT[d, p] = sum_c Watt[c, d] * k0T[c, p]  (scale folded into exp)
            kT = acts.tile([128, CT, P], BF16, tag="kT", name=f"kT{h}")
            vv = acts.tile([128, NT, C], BF16, tag="v", name=f"v{h}")
            for dt in range(CT):
                for pch in range(2):
                    pk = ps.tile([128, 512], F32, tag="mm", name=f"pk{h}{dt}{pch}")
                    for ct in range(CT):
                        nc.tensor.matmul(pk, wa_h[:, ct, dt * 128:(dt + 1) * 128],
                                         k0T[:, ct, pch * 512:(pch + 1) * 512],
                                         start=(ct == 0), stop=(ct == CT - 1))
                    nc.vector.tensor_copy(kT[:, dt, pch * 512:(pch + 1) * 512], pk)
            # v[p, c] = sum_c' source[p, c'] * Wv[c', c]
            for pt in range(NT):
                pv = ps.tile([128, 512], F32, tag="mm", name=f"pv{h}{pt}")
                for ct in range(CT):
                    nc.tensor.matmul(pv, sT[:, ct, pt * 128:(pt + 1) * 128],
                                     wv_h[:, ct, :],
                                     start=(ct == 0), stop=(ct == CT - 1))
                nc.vector.tensor_copy(vv[:, pt, :], pv)

            y = acts.tile([128, CT, N], BF16, tag="y", bufs=2, name=f"y{h}")
            for nch in range(2):
                nsl = slice(nch * 512, (nch + 1) * 512)
                # logits^T[p, n] then exp((q.k)/C) -> expT
                expT = acts.tile([128, NT, 512], BF16, tag="expT", bufs=2,
                                 name=f"expT{h}{nch}")
                for pt in range(NT):
                    pl = ps.tile([128, 512], F32, tag="mm", name=f"pl{h}{nch}{pt}")
                    for dt in range(CT):
                        nc.tensor.matmul(pl, kT[:, dt, pt * 128:(pt + 1) * 128],
                                         qT[:, dt, nsl],
                                         start=(dt == 0), stop=(dt == CT - 1))
                    nc.scalar.activation(expT[:, pt, :], pl, AF.Exp, scale=INV_C)
                # Z[n] = sum_p expT[p, n] via ones-matmul, then 1/Z broadcast
                pz = ps.tile([1, 512], F32, tag="z", bufs=2, name=f"pz{h}{nch}")
                for pt in range(NT):
                    nc.tensor.matmul(pz, ones_col, expT[:, pt, :],
                                     start=(pt == 0), stop=(pt == NT - 1))
                rz = sm.tile([1, 512], F32, tag="rz", bufs=2)
                nc.vector.reciprocal(out=rz, in_=pz)
                rz_bf = sm.tile([1, 512], BF16, tag="rzbf", bufs=2)
                nc.scalar.copy(rz_bf, rz)
                pb = ps.tile([128, 512], F32, tag="mm", name=f"pb{h}{nch}")
                nc.tensor.matmul(pb, ones_row, rz_bf, start=True, stop=True)
                rzb = sm.tile([128, 512], F32, tag="rzb", bufs=2)
                nc.vector.tensor_copy(rzb, pb)
                # out_h^T[c, n] = sum_p v[p, c] * expT[p, n]; normalize + ELU
                for ct2 in range(CT):
                    po = ps.tile([128, 512], F32, tag="mm", name=f"po{h}{nch}{ct2}")
                    for pt in range(NT):
                        nc.tensor.matmul(po, vv[:, pt, ct2 * 128:(ct2 + 1) * 128],
                                         expT[:, pt, :],
                                         start=(pt == 0), stop=(pt == NT - 1))
                    norm = sm.tile([128, 512], F32, tag="norm")
                    nc.vector.tensor_mul(norm, po, rzb)
                    e_t = sm.tile([128, 512], F32, tag="e")
                    nc.scalar.activation(e_t, norm, AF.Exp)
                    m_t = sm.tile([128, 512], F32, tag="m")
                    nc.vector.tensor_scalar(m_t, e_t, 1.0, -1.0,
                                            op0=ALU.min, op1=ALU.add)
                    r_t = sm.tile([128, 512], F32, tag="r")
                    nc.scalar.activation(r_t, norm, AF.Relu)
                    nc.vector.tensor_add(y[:, ct2, nsl], r_t, m_t)

            # partial W_O: wo_acc[n, c_out] += sum_hc y[hc, n] * Wo[hc, c_out]
            for nt in range(NT):
                pw = ps.tile([128, 512], F32, tag="mm", name=f"pw{h}{nt}")
                for ct2 in range(CT):
                    nc.tensor.matmul(pw, y[:, ct2, nt * 128:(nt + 1) * 128],
                                     wo_h[:, ct2, :],
                                     start=(ct2 == 0), stop=(ct2 == CT - 1))
                if h == 0:
                    nc.vector.tensor_copy(wo_acc[:, nt, :], pw)
                else:
                    nc.vector.tensor_add(wo_acc[:, nt, :], wo_acc[:, nt, :], pw)

        # ---------- tail: pairwise ReduceScatter + residual ----------
        partial = dram.tile([N, C], F32)
        nc.sync.dma_start(out=_r(partial[:]), in_=wo_acc)
        rs_out = dram.tile([N // 2, C], F32)
        nc.gpsimd.collective_compute(
            "ReduceScatter", ALU.add,
            replica_groups=[[0, 1], [2, 3], [4, 5], [6, 7]],
            ins=[partial[:]], outs=[rs_out[:]])
        rs_sb = acts.tile([128, NT // 2, C], F32, tag="qT", bufs=2, name="rs_sb")
        nc.sync.dma_start(out=rs_sb, in_=_r(rs_out[:]))
        res_sb = acts.tile([128, NT // 2, C], F32, tag="k0T", name="res_sb")
        nc.sync.dma_start(out=res_sb, in_=_r(resid[:]))
        for nt in range(NT // 2):
            nc.vector.tensor_add(rs_sb[:, nt, :], rs_sb[:, nt, :], res_sb[:, nt, :])
        nc.sync.dma_start(out=_r(out_d[:]), in_=rs_sb)

    return nc


_CACHED = {}


def _get_nc():
    if "nc" not in _CACHED:
        _CACHED["nc"] = build()
    return _CACHED["nc"]


def _in_maps(target, source, ln_g, ln_b, Wq, Wk, Wv, W_att, Wo):
    bf = lambda x: np.ascontiguousarray(x).astype(ml_dtypes.bfloat16)
    f = lambda x: np.ascontiguousarray(x, dtype=np.float32)
    maps = []
    for c in range(8):
        b, g = c // 2, c % 2
        hs = slice(g * HL * C, (g + 1) * HL * C)
        maps.append({
            "target": f(target[b]),
            "resid": f(target[b, g * (N // 2):(g + 1) * (N // 2)]),
            "src_bf": bf(source[b]),
            "ln_g": f(ln_g),
            "ln_b": f(ln_b),
            "wq": bf(Wq[:, hs]),
            "wk": bf(Wk[:, hs]),
            "wv": bf(Wv[:, hs]),
            "watt": bf(W_att[g * HL:(g + 1) * HL]),
            "wo": bf(Wo[hs, :]),
        })
    return maps


def _run(inputs, **kw):
    maps = _in_maps(**{k: np.asarray(v) for k, v in inputs.items()})
    res = run_bass_kernel_spmd(_get_nc(), maps, core_ids=list(range(8)), **kw)
    out = np.empty((B, N, C), np.float32)
    for c in range(8):
        b, g = c // 2, c % 2
        out[b, g * (N // 2):(g + 1) * (N // 2)] = res.results[c]["out"]
    return out, res


def kernel(**inputs) -> np.ndarray:
    out, _ = _run(inputs)
    return out

